# revision 21
# baseline (speedup 1.0000x reference)
"""Causal self-attention (GQA + RoPE) Trainium2 Bass kernel, 8-core SPMD.

Problem shapes (hardcoded): B=2, S=2048, D=1024, NH=16 q-heads, KVH=4
kv-heads, HD=64, RoPE base 10000, fp32 I/O.

Sharding (batch x kv-group): core c -> batch b = c//4, kv-group g = c%4.
Each kv-group owns one kv head and its 4 q heads (GQA repeat=4), so the
whole causal attention for those heads is local to the core. Each core
computes the partial output projection y_g @ Wo[g-block]; the host sums
the 4 partials per batch ("gather/unshard").

Per-core device kernel (bf16 activations end to end):
  inputs : xT [1024, 2048] bf16 (x[b] transposed+cast on host),
           wq [1024, 256] bf16, wkv [1024, 128] bf16 (Wk_g ++ Wv_g),
           wo [256, 1024] f32
  output : out [2048, 2048] bf16 partial (host upcasts + sums)

Structure: the x DMA, QKV projections, RoPE, and V transposes are
pipelined per 512-token s-block against the causal attention of the
previous block (attention for query tile qt only needs keys/values
0..qt), so the projection phase hides under attention instead of
serializing in front of it.  Scores for the two head-halves of a pair
run concurrently on the PE array via K=64 row tiling (partitions 0-63 /
64-127).  exp() is trimmed to skip fully-masked key chunks' columns.
Softmax denominator rides the PV matmul as a ones-column (row 64 of
y_ps); division = broadcast denominator via K=1 matmul (col-tiled pair),
one full-width reciprocal, and two muls written straight into Y.
"""
import numpy as np
from contextlib import ExitStack

import ml_dtypes

import concourse.bass as bass
import concourse.tile as tile
from concourse import bacc, mybir
from concourse.bass_utils import run_bass_kernel_spmd

F32 = mybir.dt.float32
F32R = mybir.dt.float32r
BF16 = mybir.dt.bfloat16
AF = mybir.ActivationFunctionType
BFNP = ml_dtypes.bfloat16

B, S, D = 2, 2048, 1024
NH, KVH, HD = 16, 4, 64
N_CORES = 8
SCALE = HD ** -0.5  # 0.125

DT_ATTN = BF16

_CACHE = {}


def _rope_tables():
    half = HD // 2
    inv_freq = (1.0 / (10000.0 ** (np.arange(half, dtype=np.float32) / half))
                ).astype(np.float32)
    t = np.arange(S, dtype=np.float32)
    freqs = np.outer(t, inv_freq).astype(np.float32)      # [S, 32]
    emb = np.concatenate([freqs, freqs], axis=1)          # [S, 64]
    cos_T = np.cos(emb).T.astype(np.float32)              # [64, S]
    sin_T = np.sin(emb).T.astype(np.float32)
    sin_n = sin_T.copy()
    sin_n[:half] *= -1.0                                  # sign for rotate_half
    cos4 = np.tile(cos_T, (2, 1)).astype(BFNP)            # [128, S] (2 heads)
    sin4 = np.tile(sin_n, (2, 1)).astype(BFNP)
    return cos4, sin4


def _build_kernel():
    nc = bacc.Bacc("TRN2", target_bir_lowering=False, debug=False,
                   num_devices=N_CORES)

    # xTb: per-s-block contiguous layout [sb, p, kc*512] so each block's
    # DMA is one 8KB-contiguous descriptor per partition.
    xTb_ap = nc.dram_tensor("xTb", [4, 128, 8 * 512], BF16,
                            kind="ExternalInput").ap()
    wq_ap = nc.dram_tensor("wq", [D, 256], BF16, kind="ExternalInput").ap()
    wkv_ap = nc.dram_tensor("wkv", [D, 128], BF16, kind="ExternalInput").ap()
    wo_ap = nc.dram_tensor("wo", [256, D], BF16, kind="ExternalInput").ap()
    out_ap = nc.dram_tensor("out", [S, D], BF16, kind="ExternalOutput").ap()

    cos4_np, sin4_np = _rope_tables()
    cos4_d = nc.inline_tensor(cos4_np, name="cos4").ap()
    sin4_d = nc.inline_tensor(sin4_np, name="sin4").ap()
    tri_np = (np.arange(128)[None, :] >= np.arange(128)[:, None]
              ).astype(BFNP)                              # [k, q] keep q>=k
    tri_d = nc.inline_tensor(tri_np, name="tri").ap()
    # ones row parked at partition 64 so the K=1 denominator-broadcast
    # matmul's lhsT/rhs base partitions match (denominator lives in row 64
    # of ysb).
    ones_hi_np = np.zeros((65, 64), np.float32)
    ones_hi_np[64] = 1.0
    ones_hi_d = nc.inline_tensor(ones_hi_np, name="ones_hi").ap()
    ones16_d = nc.inline_tensor(np.ones((128, 16), BFNP), name="ones16").ap()
    id_np = np.zeros((128, 64), BFNP)
    id_np[64:128] = np.eye(64, dtype=BFNP)
    id_d = nc.inline_tensor(id_np, name="id64").ap()

    with tile.TileContext(nc) as tc, ExitStack() as top:
        # ---- constants + persistent sbuf tiles -------------------------
        const = top.enter_context(tc.tile_pool(name="const", bufs=1))
        cos4 = const.tile([128, S], DT_ATTN, tag="cos4")
        sin4 = const.tile([128, S], DT_ATTN, tag="sin4")
        tri = const.tile([128, 128], DT_ATTN, tag="tri")
        ones_hi = const.tile([65, 64], F32R, tag="ones_hi")
        id64 = const.tile([128, 64], DT_ATTN, tag="id64")

        wpool = top.enter_context(tc.tile_pool(name="w", bufs=1))
        wq_sb = wpool.tile([128, 8 * 256], DT_ATTN, tag="wq")
        wkv_sb = wpool.tile([128, 8 * 128], DT_ATTN, tag="wkv")
        wo_sb = wpool.tile([128, 2 * 1024], DT_ATTN, tag="wo")

        act = top.enter_context(tc.tile_pool(name="acts", bufs=1))
        qp = [act.tile([128, S], DT_ATTN, tag=f"qp{i}", name=f"qp{i}")
              for i in range(2)]
        kk = act.tile([128, S], DT_ATTN, tag="kk")
        v_all = act.tile([128, 16 * 80], DT_ATTN, tag="v_all")
        Y = [act.tile([128, S], DT_ATTN, tag=f"Y{i}", name=f"Y{i}")
             for i in range(2)]

        raw = top.enter_context(tc.tile_pool(name="raw", bufs=1))
        kvraw = raw.tile([128, S], DT_ATTN, tag="kvraw")
        qraw = [raw.tile([128, S], DT_ATTN, tag=f"qraw{i}", name=f"qraw{i}")
                for i in range(2)]
        qsh = [raw.tile([128, S], DT_ATTN, tag=f"qsh{i}", name=f"qsh{i}")
               for i in range(2)]
        ksh = raw.tile([64, S], DT_ATTN, tag="ksh")

        xpool = top.enter_context(tc.tile_pool(name="xT", bufs=2))

        # PSUM: scores 2x[128,1024] (4 banks) + y 2x[65,512] (2 banks)
        # + pool A (proj accum + denominator broadcast, 1 bank) + pool B
        # (v transposes + outproj, 1 bank) = 8 banks.  A and B are split
        # so outproj(qt) tiles don't queue behind attn(qt)'s bc tiles and
        # proj(qt+1) can overlap outproj(qt).
        spool = top.enter_context(tc.tile_pool(name="sc", bufs=2, space="PSUM"))
        ypool = top.enter_context(tc.tile_pool(name="yps", bufs=2, space="PSUM"))
        apool = top.enter_context(tc.tile_pool(name="pa", bufs=1, space="PSUM"))
        bpool = top.enter_context(tc.tile_pool(name="pb", bufs=1, space="PSUM"))

        epool = top.enter_context(tc.tile_pool(name="ex", bufs=3))
        dpool = top.enter_context(tc.tile_pool(name="div", bufs=2))
        osb = top.enter_context(tc.tile_pool(name="osb", bufs=3))

        # ---- prologue DMAs --------------------------------------------
        # x block 0 first (longest pole), then weights, x1, wo on the sync
        # queue; constants on the gpsimd queue run concurrently.
        xb = [None] * 4

        def load_x(sb, split=False):
            t = xpool.tile([128, 8 * 512], DT_ATTN, tag="xb", name=f"xb{sb}")
            if split:
                nc.sync.dma_start(t[:, 0:2048], xTb_ap[sb][:, 0:2048])
                nc.sync.dma_start(t[:, 2048:4096], xTb_ap[sb][:, 2048:4096])
            else:
                nc.sync.dma_start(t[:], xTb_ap[sb])
            xb[sb] = t

        load_x(0, split=True)
        nc.sync.dma_start(wkv_sb[:].rearrange("p (kc m) -> p kc m", kc=8),
                          wkv_ap.rearrange("(kc p) m -> p kc m", p=128))
        nc.sync.dma_start(wq_sb[:].rearrange("p (kc m) -> p kc m", kc=8),
                          wq_ap.rearrange("(kc p) m -> p kc m", p=128))
        nc.gpsimd.dma_start(cos4[:], cos4_d[:])
        nc.gpsimd.dma_start(sin4[:], sin4_d[:])
        nc.gpsimd.dma_start(tri[:], tri_d[:])
        nc.gpsimd.dma_start(ones_hi[:], ones_hi_d[:])
        nc.gpsimd.dma_start(id64[:], id_d[:])
        ones_cols = v_all[:].rearrange("p (s c) -> p s c", c=80)[:, :, 64]
        nc.gpsimd.dma_start(ones_cols, ones16_d[:])

        def proj(sb):
            cs = slice(sb * 512, (sb + 1) * 512)
            with nc.named_scope(f"proj{sb}"):
                ps = apool.tile([128, 512], F32, tag="a", name=f"kvp{sb}")
                for kc in range(8):
                    nc.tensor.matmul(ps[:], wkv_sb[:, kc * 128:(kc + 1) * 128],
                                     xb[sb][:, kc * 512:(kc + 1) * 512],
                                     start=(kc == 0), stop=(kc == 7))
                nc.vector.tensor_copy(kvraw[:, cs], ps[:])
                for i in range(2):
                    ps = apool.tile([128, 512], F32, tag="a", name=f"q{i}p{sb}")
                    for kc in range(8):
                        nc.tensor.matmul(
                            ps[:],
                            wq_sb[:, kc * 256 + i * 128:kc * 256 + (i + 1) * 128],
                            xb[sb][:, kc * 512:(kc + 1) * 512],
                            start=(kc == 0), stop=(kc == 7))
                    nc.vector.tensor_copy(qraw[i][:, cs], ps[:])

        def rope(sb):
            cs = slice(sb * 512, (sb + 1) * 512)
            with nc.named_scope(f"rope{sb}"):
                nc.sync.dma_start(ksh[0:32, cs], kvraw[32:64, cs])
                nc.sync.dma_start(ksh[32:64, cs], kvraw[0:32, cs])
                nc.vector.tensor_mul(kk[0:64, cs], kvraw[0:64, cs], cos4[0:64, cs])
                nc.vector.tensor_mul(ksh[:, cs], ksh[:, cs], sin4[0:64, cs])
                nc.vector.tensor_add(kk[0:64, cs], kk[0:64, cs], ksh[:, cs])
                nc.sync.dma_start(kk[64:128, cs], kk[0:64, cs])
                for i in range(2):
                    for h in range(2):
                        hb = h * 64
                        nc.sync.dma_start(qsh[i][hb:hb + 32, cs],
                                          qraw[i][hb + 32:hb + 64, cs])
                        nc.sync.dma_start(qsh[i][hb + 32:hb + 64, cs],
                                          qraw[i][hb:hb + 32, cs])
                    nc.vector.tensor_mul(qp[i][:, cs], qraw[i][:, cs], cos4[:, cs])
                    nc.vector.tensor_mul(qsh[i][:, cs], qsh[i][:, cs], sin4[:, cs])
                    nc.vector.tensor_add(qp[i][:, cs], qp[i][:, cs], qsh[i][:, cs])

        def vprep(sb):
            with nc.named_scope(f"vprep{sb}"):
                for st in range(4 * sb, 4 * sb + 4):
                    tp = bpool.tile([128, 512], DT_ATTN, tag="b", name=f"vt{st}")
                    nc.tensor.transpose(
                        tp[:, 0:64], kvraw[64:128, st * 128:(st + 1) * 128],
                        id64[64:128, :])
                    nc.vector.tensor_copy(v_all[:, st * 80:st * 80 + 64],
                                          tp[:, 0:64])

        def attn(qt):
            nkc = 4 * qt + 4
            with nc.named_scope(f"attn{qt}"):
                for pair in range(2):
                    y_ps = [ypool.tile([65, 512], F32, tag="y",
                                       name=f"y{pair}{qt}{h}") for h in range(2)]
                    for kc in range(nkc):
                        j = kc - 4 * qt
                        off = j * 128 if 0 <= j < 4 else 0
                        sc = spool.tile([128, 1024], F32, tag="sc",
                                        name=f"sc{pair}{qt}{kc}")
                        for hl in range(2):
                            hb = hl * 64
                            nc.tensor.matmul(
                                sc[:, hl * 512 + off:(hl + 1) * 512],
                                kk[hb:hb + 64, kc * 128:(kc + 1) * 128],
                                qp[pair][hb:hb + 64,
                                         qt * 512 + off:(qt + 1) * 512],
                                start=True, stop=True)
                        ex = epool.tile([128, 1024], DT_ATTN, tag="ex",
                                        name=f"ex{pair}{qt}{kc}")
                        # single instruction over [off, 1024); the unwritten
                        # PSUM gap [512, 512+off) exps garbage into an
                        # unread ex region (never tri-masked or PV-read).
                        nc.scalar.activation(ex[:, off:1024], sc[:, off:1024],
                                             AF.Exp, scale=SCALE)
                        if 0 <= j < 4:
                            ex3 = ex[:].rearrange("p (b x) -> p b x",
                                                  b=2)[:, :, off:off + 128]
                            tri3 = tri[:, None, :].broadcast_to([128, 2, 128])
                            nc.vector.tensor_mul(ex3, ex3, tri3)
                        for hl in range(2):
                            nc.tensor.matmul(
                                y_ps[hl][:, off:512],
                                v_all[:, kc * 80:kc * 80 + 65],
                                ex[:, hl * 512 + off:(hl + 1) * 512],
                                start=(kc == 0), stop=(kc == nkc - 1))
                    # division: ysb <- y_ps (frees banks); broadcast raw
                    # denominator (row 64) via K=1 matmul pair (col-tiled);
                    # reciprocal at full width; two muls straight into Y.
                    ysb = dpool.tile([65, 1024], F32R, tag="ysb")
                    for hl in range(2):
                        nc.vector.tensor_copy(ysb[:, hl * 512:(hl + 1) * 512],
                                              y_ps[hl][:])
                    den_r = ysb[64:65, :]
                    bcsb = dpool.tile([64, 1024], F32, tag="bcsb")
                    ytmp = dpool.tile([64, 1024], DT_ATTN, tag="ytmp")
                    for hl in range(2):
                        bc_ps = apool.tile([64, 512], F32, tag="a",
                                           name=f"bc{pair}{qt}{hl}")
                        nc.tensor.matmul(bc_ps[:], ones_hi[64:65, :],
                                         den_r[:, hl * 512:(hl + 1) * 512],
                                         start=True, stop=True)
                        nc.vector.reciprocal_approx_fast(
                            bcsb[:, hl * 512:(hl + 1) * 512], bc_ps[:])
                        nc.vector.tensor_mul(
                            ytmp[:, hl * 512:(hl + 1) * 512],
                            ysb[0:64, hl * 512:(hl + 1) * 512],
                            bcsb[:, hl * 512:(hl + 1) * 512])
                        nc.sync.dma_start(
                            Y[pair][hl * 64:(hl + 1) * 64,
                                    qt * 512:(qt + 1) * 512],
                            ytmp[:, hl * 512:(hl + 1) * 512])

        def outproj(qt):
            with nc.named_scope(f"outproj{qt}"):
                for st in range(4 * qt, 4 * qt + 4):
                    for nt in range(2):
                        po = bpool.tile([128, 512], F32, tag="b",
                                        name=f"po{st}{nt}")
                        for cc in range(2):
                            nc.tensor.matmul(
                                po[:],
                                Y[cc][:, st * 128:(st + 1) * 128],
                                wo_sb[:, cc * 1024 + nt * 512:
                                      cc * 1024 + (nt + 1) * 512],
                                start=(cc == 0), stop=(cc == 1))
                        ot = osb.tile([128, 512], DT_ATTN, tag="ot")
                        nc.vector.tensor_copy(ot[:], po[:])
                        nc.sync.dma_start(
                            out_ap[st * 128:(st + 1) * 128,
                                   nt * 512:(nt + 1) * 512],
                            ot[:])

        # ---- software pipeline ----------------------------------------
        # proj/rope/vprep of block qt+1 are emitted BEFORE outproj(qt) so
        # their misc-pool slot allocations rotate ahead of the po tiles
        # (slot order is allocation order) and they can overlap attn(qt).
        proj(0)
        # x1 + wo issued after proj(0) so proj(0)'s matmul waits don't
        # transitively cover these later same-queue transfers.
        load_x(1)
        nc.sync.dma_start(wo_sb[:].rearrange("p (c n) -> p c n", c=2),
                          wo_ap.rearrange("(c p) n -> p c n", p=128))
        rope(0)
        vprep(0)
        for qt in range(4):
            attn(qt)
            if qt < 3:
                if qt + 2 <= 3:
                    load_x(qt + 2)
                proj(qt + 1)
                rope(qt + 1)
                vprep(qt + 1)
            outproj(qt)

    nc.compile()
    return nc


def _shard_inputs(x, Wq, Wk, Wv, Wo):
    in_maps = []
    # xTb[sb, p, kc*512+s] = x[b].T[kc*128+p, sb*512+s]  (block-contiguous)
    xb = []
    for b in range(B):
        xT = x[b].T.astype(BFNP)                      # [1024, 2048]
        t = xT.reshape(8, 128, 4, 512).transpose(2, 1, 0, 3)
        xb.append(np.ascontiguousarray(t.reshape(4, 128, 8 * 512)))
    wq_b = [np.ascontiguousarray(Wq[:, g * 256:(g + 1) * 256]).astype(BFNP)
            for g in range(4)]
    wkv_b = [np.ascontiguousarray(np.concatenate(
        [Wk[:, g * 64:(g + 1) * 64], Wv[:, g * 64:(g + 1) * 64]],
        axis=1)).astype(BFNP) for g in range(4)]
    wo_b = [np.ascontiguousarray(Wo[g * 256:(g + 1) * 256, :]).astype(BFNP)
            for g in range(4)]
    for c in range(N_CORES):
        b, g = divmod(c, 4)
        in_maps.append({
            "xTb": xb[b],
            "wq": wq_b[g],
            "wkv": wkv_b[g],
            "wo": wo_b[g],
        })
    return in_maps


def kernel(x, Wq, Wk, Wv, Wo):
    x = np.asarray(x, dtype=np.float32)
    Wq = np.asarray(Wq, dtype=np.float32)
    Wk = np.asarray(Wk, dtype=np.float32)
    Wv = np.asarray(Wv, dtype=np.float32)
    Wo = np.asarray(Wo, dtype=np.float32)
    assert x.shape == (B, S, D), x.shape

    if "nc" not in _CACHE:
        _CACHE["nc"] = _build_kernel()
    nc = _CACHE["nc"]

    in_maps = _shard_inputs(x, Wq, Wk, Wv, Wo)
    res = run_bass_kernel_spmd(nc, in_maps, list(range(N_CORES)))

    out = np.zeros((B, S, D), dtype=np.float32)
    for c in range(N_CORES):
        out[c // 4] += np.asarray(res.results[c]["out"]).astype(np.float32)
    return out
